# revision 12
# baseline (speedup 1.0000x reference)
"""CategoricalGCNEncoder on 8 Trainium2 NeuronCores (Bass/Tile).

Design ("v8" = v7b + batched groups):
  - All matmul operands bf16 (fp32 matmul runs at 1/4 PE rate); PSUM fp32.
  - Tables stored as [rows, 128] bf16 (features + zero pad to the 256B
    dma_gather row minimum); messages arrive matmul-ready in bf16.
  - Node chunks: each core's 12500 nodes are split into 4 contiguous chunks
    of 3125, packed into 26 windows each.  Gather bucket of an edge = chunk
    of its src node, so the AllGather runs per chunk: bucket c's table region
    is exactly the concatenation of all cores' chunk-c rows.
  - Pipeline: embedding emits AllGather(c) as soon as chunk c's windows are
    done; edge phase runs bucket-major with an SBUF fp32 accumulator; the
    final bucket's pass runs the epilogues, which emit layer-2's chunked
    AllGathers.  Gather SWDGE queues are round-robined per call so all 4
    queue drains overlap.
  - Everything is processed in groups of GW=4 windows: one gather, one
    interleaved S tile [P, GW, 128, KQ] (all APs inner-stride-1 to enable
    the DVE 16-bit fast mode), one PSUM tile [P, GW, F], one batched
    accumulator add, batched epilogue elementwise ops.
  - LN normalize on the Scalar engine via activation(scale=rstd,
    bias=-mu*rstd); gamma1/beta1 folded into W2eff = diag(g1) @ W2 and
    bW2 = beta1 @ W2 (computed on device).
"""

import numpy as np
import ml_dtypes

import concourse.bass as bass
import concourse.mybir as mybir
import concourse.tile as tile
from concourse import bacc
from concourse.bass_utils import run_bass_kernel_spmd

# ---------------- problem constants (hardcoded; kernel must be self-contained)
N = 100000
E = 1600000
NF = 8
EMB = 16
IN_DIM = 128
HID = 64
OUT = 32
NCAT = 100
EPS = 1e-5

NCORE = 8
SH = N // NCORE            # 12500 nodes per core
P = 128
W = 104                    # windows per core
SLOTS = W * P              # 13312 slots per core (>= SH)
KQ = 4                     # columns per (window, bucket)
NQ = 4                     # src buckets == chunks == SWDGE queues
WCH = W // NQ              # windows per chunk (26)
NPC = SH // NQ             # nodes per chunk (3125)
CSLOT = WCH * P            # slots per chunk (3328)
COLS = W * KQ              # columns per bucket stream (416)
TOTCOL = NQ * COLS         # total columns (1664)
TOTPOS = TOTCOL * P        # total edge slots (212992)
TBL = NCORE * SLOTS        # table rows (106496)
BUCK = TBL // NQ           # bucket size (26624) < 32768
GW = 4                     # windows per gather/process group
CAP_Q = KQ * P             # 512 edge slots per (w, q)
FD = 128                   # padded feature row width (256B in bf16)

f32 = mybir.dt.float32
bf16 = mybir.dt.bfloat16
i16 = mybir.dt.int16
npbf16 = ml_dtypes.bfloat16

_CACHE = {}


# ------------------------------------------------------------------ program
def build_program():
    nc = bacc.Bacc(None, target_bir_lowering=False, debug=False,
                   num_devices=NCORE, num_swdge_queues=NQ,
                   dynamic_dma_scratch_size=65536)
    with tile.TileContext(nc) as tc:
        _build(nc, tc)
    nc.compile()
    return nc


def _build(nc, tc):
    AF = mybir.ActivationFunctionType
    ALU = mybir.AluOpType
    GROUPS = [list(range(NCORE))]

    from contextlib import ExitStack
    ctx = ExitStack()
    dram = ctx.enter_context(tc.tile_pool(name="dram", bufs=1, space="DRAM"))
    const = ctx.enter_context(tc.tile_pool(name="const", bufs=1))
    oh_pool = ctx.enter_context(tc.tile_pool(name="ohp", bufs=2))
    msg_pool = ctx.enter_context(tc.tile_pool(name="msgp", bufs=5))
    s_pool = ctx.enter_context(tc.tile_pool(name="sp", bufs=2))
    epi_pool = ctx.enter_context(tc.tile_pool(name="epip", bufs=2))
    psum_em = ctx.enter_context(tc.tile_pool(name="psem", bufs=2, space="PSUM"))
    psum_ed = ctx.enter_context(tc.tile_pool(name="psed", bufs=2, space="PSUM"))
    psum_tr = ctx.enter_context(tc.tile_pool(name="pstr", bufs=2, space="PSUM"))
    psum_w2 = ctx.enter_context(tc.tile_pool(name="psw2", bufs=2, space="PSUM"))

    def din(name, shape, dtype=f32):
        return dram.tile(shape, dtype, kind="ExternalInput", name=name,
                         uniquify=False)

    # ---- inputs
    onehot = din("onehot", [NCAT, NF, SLOTS], bf16)
    idxs = din("idxs", [P, TOTPOS // 16], i16)
    dstrel = din("dstrel", [P, TOTCOL], bf16)
    degin = din("deg", [P, W])
    embT = din("embT", [EMB, NF * NCAT])
    w1 = din("w1", [EMB, NF, HID])
    w2 = din("w2", [HID, OUT])
    b1r = din("b1r", [P, HID])
    g1col = din("g1col", [HID, 1])
    be1col = din("be1col", [HID, 1])
    b2r = din("b2r", [P, OUT])
    g2r = din("g2r", [P, OUT])
    be2r = din("be2r", [P, OUT])
    # iota interleaved for S: iotari[p, j*KQ+c] = j
    iotain = din("iotari", [P, P * KQ], bf16)
    identin = din("ident", [P, P], bf16)

    outx = dram.tile([SLOTS, OUT], f32, kind="ExternalOutput", name="outx",
                     uniquify=False)

    # per-chunk bounce + table tiles (separate tiles -> precise deps, so a
    # bucket's gathers only wait for that chunk's AllGather)
    bounce1c = [dram.tile([CSLOT, FD], bf16, name=f"bounce1c{c}")
                for c in range(NQ)]
    table1c = [dram.tile([BUCK, FD], bf16, addr_space="Shared",
                         name=f"table1c{c}") for c in range(NQ)]
    bounce2c = [dram.tile([CSLOT, FD], bf16, name=f"bounce2c{c}")
                for c in range(NQ)]
    table2c = [dram.tile([BUCK, FD], bf16, addr_space="Shared",
                         name=f"table2c{c}") for c in range(NQ)]

    # ---- static SBUF
    idx_sb = const.tile([P, TOTPOS // 16], i16)
    dstrel_sb = const.tile([P, TOTCOL], bf16)
    iota_sb = const.tile([P, P * KQ], bf16)
    nc.sync.dma_start(out=iota_sb[:], in_=iotain[:])
    ident_sb = const.tile([P, P], bf16)
    nc.sync.dma_start(out=ident_sb[:], in_=identin[:])
    w1_sb = const.tile([EMB, NF, HID], f32)
    nc.sync.dma_start(out=w1_sb[:], in_=w1[:])
    w2_sb = const.tile([HID, OUT], f32)
    nc.sync.dma_start(out=w2_sb[:], in_=w2[:])
    embT_sb = const.tile([EMB, NF * NCAT], f32)
    nc.sync.dma_start(out=embT_sb[:], in_=embT[:])
    b1_sb = const.tile([P, HID], f32)
    nc.sync.dma_start(out=b1_sb[:], in_=b1r[:])
    g1col_sb = const.tile([HID, 1], f32)
    nc.sync.dma_start(out=g1col_sb[:], in_=g1col[:])
    be1col_sb = const.tile([HID, 1], f32)
    nc.sync.dma_start(out=be1col_sb[:], in_=be1col[:])
    b2_sb = const.tile([P, OUT], f32)
    nc.sync.dma_start(out=b2_sb[:], in_=b2r[:])
    g2_sb = const.tile([P, OUT], f32)
    nc.sync.dma_start(out=g2_sb[:], in_=g2r[:])
    be2_sb = const.tile([P, OUT], f32)
    nc.sync.dma_start(out=be2_sb[:], in_=be2r[:])
    eps_sb = const.tile([P, 1], f32)
    nc.vector.memset(eps_sb[:], EPS)

    # dis = 1/sqrt(deg)
    deg_sb = const.tile([P, W], f32)
    nc.sync.dma_start(out=deg_sb[:], in_=degin[:])
    dis_sb = const.tile([P, W], f32)
    nc.scalar.activation(out=dis_sb[:], in_=deg_sb[:], func=AF.Sqrt)
    nc.vector.reciprocal(out=dis_sb[:], in_=dis_sb[:])

    # W2eff = diag(gamma1) @ W2 (bf16), bW2 = beta1 @ W2 (bf16 row)
    w2eff_sb = const.tile([HID, OUT], bf16)
    nc.vector.tensor_scalar_mul(out=w2eff_sb[:], in0=w2_sb[:],
                                scalar1=g1col_sb[:])
    pbw = psum_em.tile([P, HID], f32, space="PSUM", tag="ps")
    nc.tensor.matmul(out=pbw[0:1, :OUT], lhsT=be1col_sb[:], rhs=w2_sb[:],
                     start=True, stop=True)
    bw2_sb = const.tile([1, OUT], bf16)
    nc.vector.tensor_copy(out=bw2_sb[:], in_=pbw[0:1, :OUT])
    ones1_sb = const.tile([1, P], bf16)
    nc.vector.memset(ones1_sb[:], 1.0)

    # ---- T_f = emb_f @ W1_f  -> T_sb [NCAT, NF, HID] bf16
    T_sb = const.tile([NCAT, NF, HID], bf16)
    for f in range(NF):
        pt = psum_em.tile([P, HID], f32, space="PSUM", tag="ps")
        nc.tensor.matmul(
            out=pt[:NCAT, :],
            lhsT=embT_sb[:, f * NCAT:(f + 1) * NCAT],
            rhs=w1_sb[:, f, :],
            start=True, stop=True,
        )
        nc.vector.tensor_copy(out=T_sb[:, f, :], in_=pt[:NCAT, :])

    h1pad = const.tile([P, W, FD], bf16)
    nc.vector.memset(h1pad[:], 0.0)
    h2pad = const.tile([P, W, FD], bf16)
    nc.vector.memset(h2pad[:], 0.0)

    def allgather(c, pad, bouncec, tablec):
        nc.sync.dma_start(
            out=bouncec[c].rearrange("(w p) h -> p w h", p=P),
            in_=pad[:, c * WCH:(c + 1) * WCH, :])
        nc.gpsimd.collective_compute(
            "AllGather", mybir.AluOpType.bypass,
            replica_groups=GROUPS,
            ins=[bouncec[c][:]], outs=[tablec[c][:]],
        )

    # ---- embedding: h1pad[p, w, :HID] = dis * sum_f onehot_f_w.T @ T_f
    # AllGather chunk c as soon as its 26 windows are done.
    oh2 = None
    for w in range(W):
        if w % 2 == 0:
            oh2 = oh_pool.tile([NCAT, NF, 2 * P], bf16, tag="oh")
            eng = nc.sync if (w // 2) % 2 == 0 else nc.scalar
            eng.dma_start(out=oh2[:],
                          in_=onehot[:, :, w * P:(w + 2) * P])
        pe = psum_em.tile([P, HID], f32, space="PSUM", tag="ps")
        for f in range(NF):
            nc.tensor.matmul(
                out=pe[:],
                lhsT=oh2[:, f, (w % 2) * P:(w % 2 + 1) * P],
                rhs=T_sb[:, f, :],
                start=(f == 0), stop=(f == NF - 1),
            )
        nc.vector.tensor_scalar_mul(
            out=h1pad[:, w, :HID], in0=pe[:], scalar1=dis_sb[:, w:w + 1])
        if (w + 1) % WCH == 0:
            allgather(w // WCH, h1pad, bounce1c, table1c)
            if w + 1 == WCH:
                # edge-phase index tables load during AllGather(0)'s latency
                nc.sync.dma_start(out=idx_sb[:], in_=idxs[:])
                nc.sync.dma_start(out=dstrel_sb[:], in_=dstrel[:])

    accu = const.tile([P, W, HID], f32, name="accu")

    def edge_layer(tables, fdim, selfpad, epilogue, post_epi=None):
        """Bucket-major gather + segment-sum into SBUF accumulator,
        processed in groups of GW windows (accu reused across layers)."""
        acc = accu[:, :, :fdim]
        ngrp = W // GW
        for cc in range(NQ):
            next_c = 0
            for g in range(ngrp):
                w0 = g * GW
                m = msg_pool.tile([P, GW * KQ, FD], bf16, tag="msg")
                c0 = (cc * W + w0) * KQ     # first column of chunk
                nc.gpsimd.dma_gather(
                    m[:], tables[cc][:],
                    idx_sb[:, c0 * 8:(c0 + GW * KQ) * 8],
                    num_idxs=GW * KQ * P, num_idxs_reg=GW * KQ * P,
                    elem_size=FD, single_packet=False,
                    queue_num=(cc * ngrp + g) % NQ,
                )
                # S[p, w, j, c] = (dstrel[p, col(cc,w,c)] == j); interleaved
                # layout keeps every AP inner-stride-1 (DVE fast mode).
                s = s_pool.tile([P, GW, P, KQ], bf16, tag="s")
                nc.vector.tensor_tensor(
                    out=s[:],
                    in0=iota_sb.rearrange("p (o j c) -> p o j c", o=1, j=P)
                        .to_broadcast([P, GW, P, KQ]),
                    in1=dstrel_sb
                        .rearrange("p (q w c) -> p q w c", q=NQ, w=W)
                        [:, cc, w0:w0 + GW, :]
                        .rearrange("p w (o c) -> p w o c", o=1)
                        .to_broadcast([P, GW, P, KQ]),
                    op=ALU.is_equal,
                )
                pt4 = psum_ed.tile([P, GW, fdim], f32, space="PSUM",
                                   tag="pse")
                for wi in range(GW):
                    for c in range(KQ):
                        nc.tensor.matmul(
                            out=pt4[:, wi, :],
                            lhsT=s[:, wi, :, c],
                            rhs=m[:, wi * KQ + c, :fdim],
                            start=(c == 0), stop=(c == KQ - 1),
                        )
                if cc == 0:
                    # init accumulator with self-loop term
                    nc.vector.tensor_add(
                        out=acc[:, w0:w0 + GW, :], in0=pt4[:],
                        in1=selfpad[:, w0:w0 + GW, :fdim])
                elif cc < NQ - 1:
                    nc.vector.tensor_add(
                        out=acc[:, w0:w0 + GW, :],
                        in0=acc[:, w0:w0 + GW, :], in1=pt4[:])
                else:
                    epilogue(w0, pt4, acc)
                    while (post_epi is not None and next_c < NQ
                           and (g + 1) * GW >= WCH * (next_c + 1)):
                        post_epi(next_c)
                        next_c += 1

    def bcast_w(t, fdim):
        """[P, GW] -> [P, GW, fdim] free-broadcast."""
        return t.rearrange("p (w o) -> p w o", o=1).to_broadcast(
            [P, GW, fdim])

    def bcast_f(t, fdim):
        """[P, fdim] -> [P, GW, fdim] free-broadcast."""
        return t.rearrange("p (o f) -> p o f", o=1).to_broadcast(
            [P, GW, fdim])

    def layer_norm_stats(x4, fdim, tag):
        """Batched LN stats -> (rstd4, nm4) [P, GW] tiles."""
        st4 = epi_pool.tile([P, GW, 6], f32, tag="st" + tag)
        mv4 = epi_pool.tile([P, GW, 2], f32, tag="mv" + tag)
        for wi in range(GW):
            nc.vector.bn_stats(out=st4[:, wi, :], in_=x4[:, wi, :])
            nc.vector.bn_aggr(out=mv4[:, wi, :],
                              in_=st4[:, wi:wi + 1, :])
        rstd4 = epi_pool.tile([P, GW], f32, tag="rs" + tag)
        nc.scalar.activation(out=rstd4[:], in_=mv4[:, :, 1], func=AF.Sqrt,
                             bias=eps_sb[:], scale=1.0)
        nc.vector.reciprocal(out=rstd4[:], in_=rstd4[:])
        nm4 = epi_pool.tile([P, GW], f32, tag="nm" + tag)
        nc.vector.tensor_tensor(out=nm4[:], in0=mv4[:, :, 0], in1=rstd4[:],
                                op=ALU.mult)
        nc.vector.tensor_scalar_mul(out=nm4[:], in0=nm4[:], scalar1=-1.0)
        return rstd4, nm4

    def epi1(w0, pt4, acc):
        x4 = epi_pool.tile([P, GW, HID], f32, tag="x1")
        # out1 = dis*(edge_sum + self) + b1 ; relu
        nc.vector.tensor_add(out=x4[:], in0=pt4[:], in1=acc[:, w0:w0 + GW, :])
        nc.vector.tensor_tensor(out=x4[:], in0=x4[:],
                                in1=bcast_w(dis_sb[:, w0:w0 + GW], HID),
                                op=ALU.mult)
        nc.vector.tensor_tensor(out=x4[:], in0=x4[:],
                                in1=bcast_f(b1_sb[:], HID), op=ALU.add)
        nc.scalar.activation(out=x4[:], in_=x4[:], func=AF.Relu)
        rstd4, nm4 = layer_norm_stats(x4, HID, "1")
        # per window: xhat=(x-mu)*rstd (scalar engine), transpose, W2
        pw24 = psum_w2.tile([P, GW, OUT], f32, space="PSUM", tag="w2")
        for wi in range(GW):
            xb = epi_pool.tile([P, HID], bf16, tag="xb1")
            nc.scalar.activation(out=xb[:], in_=x4[:, wi, :],
                                 func=AF.Identity,
                                 bias=nm4[:, wi:wi + 1],
                                 scale=rstd4[:, wi:wi + 1])
            ptr = psum_tr.tile([HID, P], bf16, space="PSUM", tag="tr")
            nc.tensor.transpose(out=ptr[:], in_=xb[:], identity=ident_sb[:])
            xT = epi_pool.tile([HID, P], bf16, tag="xT")
            nc.vector.tensor_copy(out=xT[:], in_=ptr[:])
            nc.tensor.matmul(out=pw24[:, wi, :], lhsT=xT[:], rhs=w2eff_sb[:],
                             start=True, stop=False, skip_group_check=True)
            nc.tensor.matmul(out=pw24[:, wi, :], lhsT=ones1_sb[:],
                             rhs=bw2_sb[:], start=False, stop=True,
                             skip_group_check=True)
        # h2pad[:, w0:w0+GW, :OUT] = dis * pw24
        nc.vector.tensor_tensor(out=h2pad[:, w0:w0 + GW, :OUT], in0=pw24[:],
                                in1=bcast_w(dis_sb[:, w0:w0 + GW], OUT),
                                op=ALU.mult)

    def epi2(w0, pt4, acc):
        x4 = epi_pool.tile([P, GW, OUT], f32, tag="x2")
        nc.vector.tensor_add(out=x4[:], in0=pt4[:], in1=acc[:, w0:w0 + GW, :])
        nc.vector.tensor_tensor(out=x4[:], in0=x4[:],
                                in1=bcast_w(dis_sb[:, w0:w0 + GW], OUT),
                                op=ALU.mult)
        nc.vector.tensor_tensor(out=x4[:], in0=x4[:],
                                in1=bcast_f(b2_sb[:], OUT), op=ALU.add)
        rstd4, nm4 = layer_norm_stats(x4, OUT, "2")
        xh4 = epi_pool.tile([P, GW, OUT], f32, tag="xh2")
        for wi in range(GW):
            nc.scalar.activation(out=xh4[:, wi, :], in_=x4[:, wi, :],
                                 func=AF.Identity,
                                 bias=nm4[:, wi:wi + 1],
                                 scale=rstd4[:, wi:wi + 1])
        nc.vector.tensor_tensor(out=xh4[:], in0=xh4[:],
                                in1=bcast_f(g2_sb[:], OUT), op=ALU.mult)
        fo = epi_pool.tile([P, GW, OUT], f32, tag="fo")
        nc.vector.tensor_tensor(out=fo[:], in0=xh4[:],
                                in1=bcast_f(be2_sb[:], OUT), op=ALU.add)
        nc.sync.dma_start(
            out=outx.rearrange("(w p) o -> p w o", p=P)[:, w0:w0 + GW, :],
            in_=fo[:])

    # ---- layer 1 (epilogues emit layer-2's chunked AllGathers)
    edge_layer(table1c, HID, h1pad, epi1,
               post_epi=lambda c: allgather(c, h2pad, bounce2c, table2c))

    # ---- layer 2
    edge_layer(table2c, OUT, h2pad, epi2)
    ctx.close()


# ------------------------------------------------------------------ host prep
def _pack_chunk(dloc, q_of_edge):
    """Assign a chunk's NPC nodes to (window, slot) with per-(w,q) capacity
    CAP_Q and <=P nodes per window.  Returns win[NPC], pslot[NPC] (window
    local to the chunk)."""
    cnt = np.zeros((NPC, NQ), np.int64)
    np.add.at(cnt, (dloc, q_of_edge), 1)
    tot = cnt.sum(1)
    order = np.argsort(-tot, kind="stable")
    fills = np.zeros((WCH, NQ), np.int64)
    counts = np.zeros(WCH, np.int64)
    win = np.zeros(NPC, np.int64)
    for n in order:
        c = cnt[n]
        ok = (counts < P) & np.all(fills + c <= CAP_Q, axis=1)
        if not ok.any():
            raise RuntimeError("window packing failed")
        load = np.where(ok[:, None], fills + c, 1 << 30).max(axis=1)
        wsel = int(np.argmin(load))
        win[n] = wsel
        fills[wsel] += c
        counts[wsel] += 1
    # slot within window: order nodes by window
    pslot = np.zeros(NPC, np.int64)
    for wsel in range(WCH):
        nodes = np.nonzero(win == wsel)[0]
        pslot[nodes] = np.arange(len(nodes))
    return win, pslot


def _host_prep(x_cat, edge_index, emb_tables, W1, b1, W2, b2,
               gamma1, beta1, gamma2, beta2):
    src = np.asarray(edge_index[0], np.int64)
    dst = np.asarray(edge_index[1], np.int64)
    deg = np.bincount(dst, minlength=N).astype(np.float64) + 1.0

    core_of = np.arange(N) // SH
    # bucket of an edge = chunk of its src node (contiguous local ranges)
    srcq = (src % SH) // NPC
    # pack every (core, chunk)'s nodes into that chunk's 26 windows
    wins = np.zeros(N, np.int64)
    pslots = np.zeros(N, np.int64)
    for k in range(NCORE):
        m = (dst // SH) == k
        dl = dst[m] - k * SH
        q = srcq[m]
        for cc in range(NQ):
            mm = (dl // NPC) == cc
            win, ps = _pack_chunk(dl[mm] - cc * NPC, q[mm])
            lo = k * SH + cc * NPC
            wins[lo:lo + NPC] = cc * WCH + win
            pslots[lo:lo + NPC] = ps
    slot_of = wins * P + pslots               # slot within owner core
    # in-bucket gather row: core-major within the chunk's table region
    brow = core_of * CSLOT + (wins % WCH) * P + pslots

    in_maps = []
    perm_slots = []
    for k in range(NCORE):
        m = (dst // SH) == k
        es, ed = src[m], dst[m] - k * SH
        ew = wins[ed + k * SH]
        ep = pslots[ed + k * SH]
        eq = srcq[m]
        # stream position: per (q, w) block of CAP_Q slots, fill in order
        gkey = eq * W + ew
        order = np.argsort(gkey, kind="stable")
        gsort = gkey[order]
        # rank within group
        start = np.searchsorted(gsort, np.arange(NQ * W))
        rank = np.arange(len(gsort)) - start[gsort]
        assert (rank < CAP_Q).all()
        pos = gsort * CAP_Q + rank
        idx16 = np.zeros(TOTPOS, np.int16)
        drel = np.full(TOTPOS, -1.0, np.float32)
        idx16[pos] = brow[es][order].astype(np.int16)
        drel[pos] = ep[order].astype(np.float32)
        # wrap idx: j -> [j%16, j//16], replicate x8 partition groups
        idxw = np.tile(idx16.reshape(-1, 16).T, (8, 1))
        drelw = np.ascontiguousarray(drel.reshape(-1, P).T).astype(npbf16)

        # onehot [NCAT, NF, SLOTS] for this core's slots
        oh = np.zeros((NCAT, NF, SLOTS), npbf16)
        sl = slot_of[k * SH:(k + 1) * SH]
        xc = np.asarray(x_cat[k * SH:(k + 1) * SH], np.int64)
        for f in range(NF):
            oh[xc[:, f], f, sl] = 1.0

        degs = np.ones(SLOTS, np.float32)
        degs[sl] = deg[k * SH:(k + 1) * SH]
        degw = np.ascontiguousarray(degs.reshape(W, P).T)

        embT = np.ascontiguousarray(
            np.asarray(emb_tables, np.float32).transpose(2, 0, 1)
            .reshape(EMB, NF * NCAT))

        rep = lambda v, d: np.broadcast_to(
            np.asarray(v, np.float32).reshape(1, d), (P, d)).copy()

        # interleaved iota: iotari[p, j*KQ+c] = j
        iotari = np.broadcast_to(
            np.repeat(np.arange(P, dtype=np.float32), KQ), (P, P * KQ))

        in_maps.append({
            "onehot": oh,
            "idxs": idxw,
            "dstrel": drelw,
            "deg": degw,
            "embT": embT,
            "w1": np.ascontiguousarray(np.asarray(W1, np.float32).reshape(NF, EMB, HID).transpose(1, 0, 2)),
            "w2": np.asarray(W2, np.float32),
            "b1r": rep(b1, HID),
            "g1col": np.ascontiguousarray(
                np.asarray(gamma1, np.float32).reshape(HID, 1)),
            "be1col": np.ascontiguousarray(
                np.asarray(beta1, np.float32).reshape(HID, 1)),
            "b2r": rep(b2, OUT), "g2r": rep(gamma2, OUT),
            "be2r": rep(beta2, OUT),
            "iotari": iotari.astype(npbf16),
            "ident": np.eye(P, dtype=np.float32).astype(npbf16),
        })
        perm_slots.append(sl)
    return in_maps, perm_slots


# ------------------------------------------------------------------ entry
def kernel(x_cat, edge_index, emb_tables, W1, b1, W2, b2,
           gamma1, beta1, gamma2, beta2, _res_hook=None):
    if "nc" not in _CACHE:
        _CACHE["nc"] = build_program()
    nc = _CACHE["nc"]
    in_maps, perm_slots = _host_prep(
        np.asarray(x_cat), np.asarray(edge_index), np.asarray(emb_tables),
        np.asarray(W1), np.asarray(b1), np.asarray(W2), np.asarray(b2),
        np.asarray(gamma1), np.asarray(beta1), np.asarray(gamma2),
        np.asarray(beta2))
    res = run_bass_kernel_spmd(nc, in_maps, list(range(NCORE)),
                               **(_res_hook or {}))
    out = np.empty((N, OUT), np.float32)
    for k in range(NCORE):
        full = res.results[k]["outx"]        # [SLOTS, OUT] slot-ordered
        out[k * SH:(k + 1) * SH] = full[perm_slots[k]]
    if _res_hook is not None:
        _res_hook["result"] = res
    return out


# revision 13
# speedup vs baseline: 1.0835x; 1.0835x over previous
"""CategoricalGCNEncoder on 8 Trainium2 NeuronCores (Bass/Tile).

Design ("v8" = v7b + batched groups):
  - All matmul operands bf16 (fp32 matmul runs at 1/4 PE rate); PSUM fp32.
  - Tables stored as [rows, 128] bf16 (features + zero pad to the 256B
    dma_gather row minimum); messages arrive matmul-ready in bf16.
  - Node chunks: each core's 12500 nodes are split into 4 contiguous chunks
    of 3125, packed into 26 windows each.  Gather bucket of an edge = chunk
    of its src node, so the AllGather runs per chunk: bucket c's table region
    is exactly the concatenation of all cores' chunk-c rows.
  - Pipeline: embedding emits AllGather(c) as soon as chunk c's windows are
    done; edge phase runs bucket-major with an SBUF fp32 accumulator; the
    final bucket's pass runs the epilogues, which emit layer-2's chunked
    AllGathers.  Gather SWDGE queues are round-robined per call so all 4
    queue drains overlap.
  - Everything is processed in groups of GW=4 windows: one gather, one
    interleaved S tile [P, GW, 128, KQ] (all APs inner-stride-1 to enable
    the DVE 16-bit fast mode), one PSUM tile [P, GW, F], one batched
    accumulator add, batched epilogue elementwise ops.
  - LN normalize on the Scalar engine via activation(scale=rstd,
    bias=-mu*rstd); gamma1/beta1 folded into W2eff = diag(g1) @ W2 and
    bW2 = beta1 @ W2 (computed on device).
"""

import numpy as np
import ml_dtypes

import concourse.bass as bass
import concourse.mybir as mybir
import concourse.tile as tile
from concourse import bacc
from concourse.bass_utils import run_bass_kernel_spmd

# ---------------- problem constants (hardcoded; kernel must be self-contained)
N = 100000
E = 1600000
NF = 8
EMB = 16
IN_DIM = 128
HID = 64
OUT = 32
NCAT = 100
EPS = 1e-5

NCORE = 8
SH = N // NCORE            # 12500 nodes per core
P = 128
W = 104                    # windows per core
SLOTS = W * P              # 13312 slots per core (>= SH)
KQ = 4                     # columns per (window, bucket)
NQ = 4                     # src buckets == chunks == SWDGE queues
WCH = W // NQ              # windows per chunk (26)
NPC = SH // NQ             # nodes per chunk (3125)
CSLOT = WCH * P            # slots per chunk (3328)
COLS = W * KQ              # columns per bucket stream (416)
TOTCOL = NQ * COLS         # total columns (1664)
TOTPOS = TOTCOL * P        # total edge slots (212992)
TBL = NCORE * SLOTS        # table rows (106496)
BUCK = TBL // NQ           # bucket size (26624) < 32768
GW = 2                     # windows per gather/process group
CAP_Q = KQ * P             # 512 edge slots per (w, q)
FD = 128                   # padded feature row width (256B in bf16)

f32 = mybir.dt.float32
bf16 = mybir.dt.bfloat16
i16 = mybir.dt.int16
npbf16 = ml_dtypes.bfloat16

_CACHE = {}


# ------------------------------------------------------------------ program
def build_program():
    nc = bacc.Bacc(None, target_bir_lowering=False, debug=False,
                   num_devices=NCORE, num_swdge_queues=NQ,
                   dynamic_dma_scratch_size=49152)
    with tile.TileContext(nc) as tc:
        _build(nc, tc)
    nc.compile()
    return nc


def _build(nc, tc):
    AF = mybir.ActivationFunctionType
    ALU = mybir.AluOpType
    GROUPS = [list(range(NCORE))]

    from contextlib import ExitStack
    ctx = ExitStack()
    dram = ctx.enter_context(tc.tile_pool(name="dram", bufs=1, space="DRAM"))
    const = ctx.enter_context(tc.tile_pool(name="const", bufs=1))
    oh_pool = ctx.enter_context(tc.tile_pool(name="ohp", bufs=3))
    msg_pool = ctx.enter_context(tc.tile_pool(name="msgp", bufs=10))
    s_pool = ctx.enter_context(tc.tile_pool(name="sp", bufs=3))
    epi_pool = ctx.enter_context(tc.tile_pool(name="epip", bufs=3))
    psum_em = ctx.enter_context(tc.tile_pool(name="psem", bufs=2, space="PSUM"))
    psum_ed = ctx.enter_context(tc.tile_pool(name="psed", bufs=2, space="PSUM"))
    psum_tr = ctx.enter_context(tc.tile_pool(name="pstr", bufs=2, space="PSUM"))
    psum_w2 = ctx.enter_context(tc.tile_pool(name="psw2", bufs=2, space="PSUM"))

    def din(name, shape, dtype=f32):
        return dram.tile(shape, dtype, kind="ExternalInput", name=name,
                         uniquify=False)

    # ---- inputs
    onehot = din("onehot", [NCAT, NF, SLOTS], bf16)
    idxs = din("idxs", [P, TOTPOS // 16], i16)
    dstrel = din("dstrel", [P, TOTCOL], bf16)
    degin = din("deg", [P, W])
    embT = din("embT", [EMB, NF * NCAT])
    w1 = din("w1", [EMB, NF, HID])
    w2 = din("w2", [HID, OUT])
    b1r = din("b1r", [P, HID])
    g1col = din("g1col", [HID, 1])
    be1col = din("be1col", [HID, 1])
    b2r = din("b2r", [P, OUT])
    g2r = din("g2r", [P, OUT])
    be2r = din("be2r", [P, OUT])
    # iota interleaved for S: iotari[p, j*KQ+c] = j
    iotain = din("iotari", [P, P * KQ], bf16)
    identin = din("ident", [P, P], bf16)

    outx = dram.tile([SLOTS, OUT], f32, kind="ExternalOutput", name="outx",
                     uniquify=False)

    # per-chunk bounce + table tiles (separate tiles -> precise deps, so a
    # bucket's gathers only wait for that chunk's AllGather)
    bounce1c = [dram.tile([CSLOT, FD], bf16, name=f"bounce1c{c}")
                for c in range(NQ)]
    table1c = [dram.tile([BUCK, FD], bf16, addr_space="Shared",
                         name=f"table1c{c}") for c in range(NQ)]
    bounce2c = [dram.tile([CSLOT, FD], bf16, name=f"bounce2c{c}")
                for c in range(NQ)]
    table2c = [dram.tile([BUCK, FD], bf16, addr_space="Shared",
                         name=f"table2c{c}") for c in range(NQ)]

    # ---- static SBUF
    idx_sb = const.tile([P, TOTPOS // 16], i16)
    dstrel_sb = const.tile([P, TOTCOL], bf16)
    iota_sb = const.tile([P, P * KQ], bf16)
    nc.sync.dma_start(out=iota_sb[:], in_=iotain[:])
    ident_sb = const.tile([P, P], bf16)
    nc.sync.dma_start(out=ident_sb[:], in_=identin[:])
    w1_sb = const.tile([EMB, NF, HID], f32)
    nc.sync.dma_start(out=w1_sb[:], in_=w1[:])
    w2_sb = const.tile([HID, OUT], f32)
    nc.sync.dma_start(out=w2_sb[:], in_=w2[:])
    embT_sb = const.tile([EMB, NF * NCAT], f32)
    nc.sync.dma_start(out=embT_sb[:], in_=embT[:])
    b1_sb = const.tile([P, HID], f32)
    nc.sync.dma_start(out=b1_sb[:], in_=b1r[:])
    g1col_sb = const.tile([HID, 1], f32)
    nc.sync.dma_start(out=g1col_sb[:], in_=g1col[:])
    be1col_sb = const.tile([HID, 1], f32)
    nc.sync.dma_start(out=be1col_sb[:], in_=be1col[:])
    b2_sb = const.tile([P, OUT], f32)
    nc.sync.dma_start(out=b2_sb[:], in_=b2r[:])
    g2_sb = const.tile([P, OUT], f32)
    nc.sync.dma_start(out=g2_sb[:], in_=g2r[:])
    be2_sb = const.tile([P, OUT], f32)
    nc.sync.dma_start(out=be2_sb[:], in_=be2r[:])
    eps_sb = const.tile([P, 1], f32)
    nc.vector.memset(eps_sb[:], EPS)

    # dis = 1/sqrt(deg)
    deg_sb = const.tile([P, W], f32)
    nc.sync.dma_start(out=deg_sb[:], in_=degin[:])
    dis_sb = const.tile([P, W], f32)
    nc.scalar.activation(out=dis_sb[:], in_=deg_sb[:], func=AF.Sqrt)
    nc.vector.reciprocal(out=dis_sb[:], in_=dis_sb[:])

    # W2eff = diag(gamma1) @ W2 (bf16), bW2 = beta1 @ W2 (bf16 row)
    w2eff_sb = const.tile([HID, OUT], bf16)
    nc.vector.tensor_scalar_mul(out=w2eff_sb[:], in0=w2_sb[:],
                                scalar1=g1col_sb[:])
    pbw = psum_em.tile([P, HID], f32, space="PSUM", tag="ps")
    nc.tensor.matmul(out=pbw[0:1, :OUT], lhsT=be1col_sb[:], rhs=w2_sb[:],
                     start=True, stop=True)
    bw2_sb = const.tile([1, OUT], bf16)
    nc.vector.tensor_copy(out=bw2_sb[:], in_=pbw[0:1, :OUT])
    ones1_sb = const.tile([1, P], bf16)
    nc.vector.memset(ones1_sb[:], 1.0)

    # ---- T_f = emb_f @ W1_f  -> T_sb [NCAT, NF, HID] bf16
    T_sb = const.tile([NCAT, NF, HID], bf16)
    for f in range(NF):
        pt = psum_em.tile([P, HID], f32, space="PSUM", tag="ps")
        nc.tensor.matmul(
            out=pt[:NCAT, :],
            lhsT=embT_sb[:, f * NCAT:(f + 1) * NCAT],
            rhs=w1_sb[:, f, :],
            start=True, stop=True,
        )
        nc.vector.tensor_copy(out=T_sb[:, f, :], in_=pt[:NCAT, :])

    h1pad = const.tile([P, W, FD], bf16)
    nc.vector.memset(h1pad[:], 0.0)
    h2pad = const.tile([P, W, FD], bf16)
    nc.vector.memset(h2pad[:], 0.0)

    def allgather(c, pad, bouncec, tablec):
        nc.sync.dma_start(
            out=bouncec[c].rearrange("(w p) h -> p w h", p=P),
            in_=pad[:, c * WCH:(c + 1) * WCH, :])
        nc.gpsimd.collective_compute(
            "AllGather", mybir.AluOpType.bypass,
            replica_groups=GROUPS,
            ins=[bouncec[c][:]], outs=[tablec[c][:]],
        )

    # ---- embedding: h1pad[p, w, :HID] = dis * sum_f onehot_f_w.T @ T_f
    # AllGather chunk c as soon as its 26 windows are done.
    oh2 = None
    for w in range(W):
        if w % 2 == 0:
            oh2 = oh_pool.tile([NCAT, NF, 2 * P], bf16, tag="oh")
            eng = nc.sync if (w // 2) % 2 == 0 else nc.scalar
            eng.dma_start(out=oh2[:],
                          in_=onehot[:, :, w * P:(w + 2) * P])
        pe = psum_em.tile([P, HID], f32, space="PSUM", tag="ps")
        for f in range(NF):
            nc.tensor.matmul(
                out=pe[:],
                lhsT=oh2[:, f, (w % 2) * P:(w % 2 + 1) * P],
                rhs=T_sb[:, f, :],
                start=(f == 0), stop=(f == NF - 1),
            )
        nc.vector.tensor_scalar_mul(
            out=h1pad[:, w, :HID], in0=pe[:], scalar1=dis_sb[:, w:w + 1])
        if (w + 1) % WCH == 0:
            allgather(w // WCH, h1pad, bounce1c, table1c)
            if w + 1 == WCH:
                # edge-phase index tables load during AllGather(0)'s latency
                nc.sync.dma_start(out=idx_sb[:], in_=idxs[:])
                nc.sync.dma_start(out=dstrel_sb[:], in_=dstrel[:])

    def edge_layer(tables, fdim, selfpad, epilogue, post_epi=None):
        """Bucket-major gather + segment-sum into SBUF accumulator,
        processed in groups of GW windows."""
        acc = const.tile([P, W, fdim], f32, name=f"acc{fdim}")
        ngrp = W // GW
        for cc in range(NQ):
            next_c = 0
            for g in range(ngrp):
                w0 = g * GW
                m = msg_pool.tile([P, GW * KQ, FD], bf16, tag="msg")
                c0 = (cc * W + w0) * KQ     # first column of chunk
                nc.gpsimd.dma_gather(
                    m[:], tables[cc][:],
                    idx_sb[:, c0 * 8:(c0 + GW * KQ) * 8],
                    num_idxs=GW * KQ * P, num_idxs_reg=GW * KQ * P,
                    elem_size=FD, single_packet=False,
                    queue_num=(cc * ngrp + g) % NQ,
                )
                # S[p, w, j, c] = (dstrel[p, col(cc,w,c)] == j); interleaved
                # layout keeps every AP inner-stride-1 (DVE fast mode).
                s = s_pool.tile([P, GW, P, KQ], bf16, tag="s")
                nc.vector.tensor_tensor(
                    out=s[:],
                    in0=iota_sb.rearrange("p (o j c) -> p o j c", o=1, j=P)
                        .to_broadcast([P, GW, P, KQ]),
                    in1=dstrel_sb
                        .rearrange("p (q w c) -> p q w c", q=NQ, w=W)
                        [:, cc, w0:w0 + GW, :]
                        .rearrange("p w (o c) -> p w o c", o=1)
                        .to_broadcast([P, GW, P, KQ]),
                    op=ALU.is_equal,
                )
                pt4 = psum_ed.tile([P, GW, fdim], f32, space="PSUM",
                                   tag="pse")
                for wi in range(GW):
                    for c in range(KQ):
                        nc.tensor.matmul(
                            out=pt4[:, wi, :],
                            lhsT=s[:, wi, :, c],
                            rhs=m[:, wi * KQ + c, :fdim],
                            start=(c == 0), stop=(c == KQ - 1),
                        )
                if cc == 0:
                    # init accumulator with self-loop term
                    nc.vector.tensor_add(
                        out=acc[:, w0:w0 + GW, :], in0=pt4[:],
                        in1=selfpad[:, w0:w0 + GW, :fdim])
                elif cc < NQ - 1:
                    nc.vector.tensor_add(
                        out=acc[:, w0:w0 + GW, :],
                        in0=acc[:, w0:w0 + GW, :], in1=pt4[:])
                else:
                    epilogue(w0, pt4, acc)
                    while (post_epi is not None and next_c < NQ
                           and (g + 1) * GW >= WCH * (next_c + 1)):
                        post_epi(next_c)
                        next_c += 1

    def bcast_w(t, fdim):
        """[P, GW] -> [P, GW, fdim] free-broadcast."""
        return t.rearrange("p (w o) -> p w o", o=1).to_broadcast(
            [P, GW, fdim])

    def bcast_f(t, fdim):
        """[P, fdim] -> [P, GW, fdim] free-broadcast."""
        return t.rearrange("p (o f) -> p o f", o=1).to_broadcast(
            [P, GW, fdim])

    def layer_norm_stats(x4, fdim, tag):
        """Batched LN stats -> (rstd4, nm4) [P, GW] tiles."""
        st4 = epi_pool.tile([P, GW, 6], f32, tag="st" + tag)
        mv4 = epi_pool.tile([P, GW, 2], f32, tag="mv" + tag)
        for wi in range(GW):
            nc.vector.bn_stats(out=st4[:, wi, :], in_=x4[:, wi, :])
            nc.vector.bn_aggr(out=mv4[:, wi, :],
                              in_=st4[:, wi:wi + 1, :])
        rstd4 = epi_pool.tile([P, GW], f32, tag="rs" + tag)
        nc.scalar.activation(out=rstd4[:], in_=mv4[:, :, 1], func=AF.Sqrt,
                             bias=eps_sb[:], scale=1.0)
        nc.vector.reciprocal(out=rstd4[:], in_=rstd4[:])
        nm4 = epi_pool.tile([P, GW], f32, tag="nm" + tag)
        nc.vector.tensor_tensor(out=nm4[:], in0=mv4[:, :, 0], in1=rstd4[:],
                                op=ALU.mult)
        nc.vector.tensor_scalar_mul(out=nm4[:], in0=nm4[:], scalar1=-1.0)
        return rstd4, nm4

    def epi1(w0, pt4, acc):
        x4 = epi_pool.tile([P, GW, HID], f32, tag="x1")
        # out1 = dis*(edge_sum + self) + b1 ; relu
        nc.vector.tensor_add(out=x4[:], in0=pt4[:], in1=acc[:, w0:w0 + GW, :])
        nc.vector.tensor_tensor(out=x4[:], in0=x4[:],
                                in1=bcast_w(dis_sb[:, w0:w0 + GW], HID),
                                op=ALU.mult)
        nc.vector.tensor_tensor(out=x4[:], in0=x4[:],
                                in1=bcast_f(b1_sb[:], HID), op=ALU.add)
        nc.scalar.activation(out=x4[:], in_=x4[:], func=AF.Relu)
        rstd4, nm4 = layer_norm_stats(x4, HID, "1")
        # per window: xhat=(x-mu)*rstd (scalar engine), transpose, W2
        pw24 = psum_w2.tile([P, GW, OUT], f32, space="PSUM", tag="w2")
        for wi in range(GW):
            xb = epi_pool.tile([P, HID], bf16, tag="xb1")
            nc.scalar.activation(out=xb[:], in_=x4[:, wi, :],
                                 func=AF.Identity,
                                 bias=nm4[:, wi:wi + 1],
                                 scale=rstd4[:, wi:wi + 1])
            ptr = psum_tr.tile([HID, P], bf16, space="PSUM", tag="tr")
            nc.tensor.transpose(out=ptr[:], in_=xb[:], identity=ident_sb[:])
            xT = epi_pool.tile([HID, P], bf16, tag="xT")
            nc.vector.tensor_copy(out=xT[:], in_=ptr[:])
            nc.tensor.matmul(out=pw24[:, wi, :], lhsT=xT[:], rhs=w2eff_sb[:],
                             start=True, stop=False, skip_group_check=True)
            nc.tensor.matmul(out=pw24[:, wi, :], lhsT=ones1_sb[:],
                             rhs=bw2_sb[:], start=False, stop=True,
                             skip_group_check=True)
        # h2pad[:, w0:w0+GW, :OUT] = dis * pw24
        nc.vector.tensor_tensor(out=h2pad[:, w0:w0 + GW, :OUT], in0=pw24[:],
                                in1=bcast_w(dis_sb[:, w0:w0 + GW], OUT),
                                op=ALU.mult)

    def epi2(w0, pt4, acc):
        x4 = epi_pool.tile([P, GW, OUT], f32, tag="x2")
        nc.vector.tensor_add(out=x4[:], in0=pt4[:], in1=acc[:, w0:w0 + GW, :])
        nc.vector.tensor_tensor(out=x4[:], in0=x4[:],
                                in1=bcast_w(dis_sb[:, w0:w0 + GW], OUT),
                                op=ALU.mult)
        nc.vector.tensor_tensor(out=x4[:], in0=x4[:],
                                in1=bcast_f(b2_sb[:], OUT), op=ALU.add)
        rstd4, nm4 = layer_norm_stats(x4, OUT, "2")
        xh4 = epi_pool.tile([P, GW, OUT], f32, tag="xh2")
        for wi in range(GW):
            nc.scalar.activation(out=xh4[:, wi, :], in_=x4[:, wi, :],
                                 func=AF.Identity,
                                 bias=nm4[:, wi:wi + 1],
                                 scale=rstd4[:, wi:wi + 1])
        nc.vector.tensor_tensor(out=xh4[:], in0=xh4[:],
                                in1=bcast_f(g2_sb[:], OUT), op=ALU.mult)
        fo = epi_pool.tile([P, GW, OUT], f32, tag="fo")
        nc.vector.tensor_tensor(out=fo[:], in0=xh4[:],
                                in1=bcast_f(be2_sb[:], OUT), op=ALU.add)
        nc.sync.dma_start(
            out=outx.rearrange("(w p) o -> p w o", p=P)[:, w0:w0 + GW, :],
            in_=fo[:])

    # ---- layer 1 (epilogues emit layer-2's chunked AllGathers)
    edge_layer(table1c, HID, h1pad, epi1,
               post_epi=lambda c: allgather(c, h2pad, bounce2c, table2c))

    # ---- layer 2
    edge_layer(table2c, OUT, h2pad, epi2)
    ctx.close()


# ------------------------------------------------------------------ host prep
def _pack_chunk(dloc, q_of_edge):
    """Assign a chunk's NPC nodes to (window, slot) with per-(w,q) capacity
    CAP_Q and <=P nodes per window.  Returns win[NPC], pslot[NPC] (window
    local to the chunk)."""
    cnt = np.zeros((NPC, NQ), np.int64)
    np.add.at(cnt, (dloc, q_of_edge), 1)
    tot = cnt.sum(1)
    order = np.argsort(-tot, kind="stable")
    fills = np.zeros((WCH, NQ), np.int64)
    counts = np.zeros(WCH, np.int64)
    win = np.zeros(NPC, np.int64)
    for n in order:
        c = cnt[n]
        ok = (counts < P) & np.all(fills + c <= CAP_Q, axis=1)
        if not ok.any():
            raise RuntimeError("window packing failed")
        load = np.where(ok[:, None], fills + c, 1 << 30).max(axis=1)
        wsel = int(np.argmin(load))
        win[n] = wsel
        fills[wsel] += c
        counts[wsel] += 1
    # slot within window: order nodes by window
    pslot = np.zeros(NPC, np.int64)
    for wsel in range(WCH):
        nodes = np.nonzero(win == wsel)[0]
        pslot[nodes] = np.arange(len(nodes))
    return win, pslot


def _host_prep(x_cat, edge_index, emb_tables, W1, b1, W2, b2,
               gamma1, beta1, gamma2, beta2):
    src = np.asarray(edge_index[0], np.int64)
    dst = np.asarray(edge_index[1], np.int64)
    deg = np.bincount(dst, minlength=N).astype(np.float64) + 1.0

    core_of = np.arange(N) // SH
    # bucket of an edge = chunk of its src node (contiguous local ranges)
    srcq = (src % SH) // NPC
    # pack every (core, chunk)'s nodes into that chunk's 26 windows
    wins = np.zeros(N, np.int64)
    pslots = np.zeros(N, np.int64)
    for k in range(NCORE):
        m = (dst // SH) == k
        dl = dst[m] - k * SH
        q = srcq[m]
        for cc in range(NQ):
            mm = (dl // NPC) == cc
            win, ps = _pack_chunk(dl[mm] - cc * NPC, q[mm])
            lo = k * SH + cc * NPC
            wins[lo:lo + NPC] = cc * WCH + win
            pslots[lo:lo + NPC] = ps
    slot_of = wins * P + pslots               # slot within owner core
    # in-bucket gather row: core-major within the chunk's table region
    brow = core_of * CSLOT + (wins % WCH) * P + pslots

    in_maps = []
    perm_slots = []
    for k in range(NCORE):
        m = (dst // SH) == k
        es, ed = src[m], dst[m] - k * SH
        ew = wins[ed + k * SH]
        ep = pslots[ed + k * SH]
        eq = srcq[m]
        # stream position: per (q, w) block of CAP_Q slots, fill in order
        gkey = eq * W + ew
        order = np.argsort(gkey, kind="stable")
        gsort = gkey[order]
        # rank within group
        start = np.searchsorted(gsort, np.arange(NQ * W))
        rank = np.arange(len(gsort)) - start[gsort]
        assert (rank < CAP_Q).all()
        pos = gsort * CAP_Q + rank
        idx16 = np.zeros(TOTPOS, np.int16)
        drel = np.full(TOTPOS, -1.0, np.float32)
        idx16[pos] = brow[es][order].astype(np.int16)
        drel[pos] = ep[order].astype(np.float32)
        # wrap idx: j -> [j%16, j//16], replicate x8 partition groups
        idxw = np.tile(idx16.reshape(-1, 16).T, (8, 1))
        drelw = np.ascontiguousarray(drel.reshape(-1, P).T).astype(npbf16)

        # onehot [NCAT, NF, SLOTS] for this core's slots
        oh = np.zeros((NCAT, NF, SLOTS), npbf16)
        sl = slot_of[k * SH:(k + 1) * SH]
        xc = np.asarray(x_cat[k * SH:(k + 1) * SH], np.int64)
        for f in range(NF):
            oh[xc[:, f], f, sl] = 1.0

        degs = np.ones(SLOTS, np.float32)
        degs[sl] = deg[k * SH:(k + 1) * SH]
        degw = np.ascontiguousarray(degs.reshape(W, P).T)

        embT = np.ascontiguousarray(
            np.asarray(emb_tables, np.float32).transpose(2, 0, 1)
            .reshape(EMB, NF * NCAT))

        rep = lambda v, d: np.broadcast_to(
            np.asarray(v, np.float32).reshape(1, d), (P, d)).copy()

        # interleaved iota: iotari[p, j*KQ+c] = j
        iotari = np.broadcast_to(
            np.repeat(np.arange(P, dtype=np.float32), KQ), (P, P * KQ))

        in_maps.append({
            "onehot": oh,
            "idxs": idxw,
            "dstrel": drelw,
            "deg": degw,
            "embT": embT,
            "w1": np.ascontiguousarray(np.asarray(W1, np.float32).reshape(NF, EMB, HID).transpose(1, 0, 2)),
            "w2": np.asarray(W2, np.float32),
            "b1r": rep(b1, HID),
            "g1col": np.ascontiguousarray(
                np.asarray(gamma1, np.float32).reshape(HID, 1)),
            "be1col": np.ascontiguousarray(
                np.asarray(beta1, np.float32).reshape(HID, 1)),
            "b2r": rep(b2, OUT), "g2r": rep(gamma2, OUT),
            "be2r": rep(beta2, OUT),
            "iotari": iotari.astype(npbf16),
            "ident": np.eye(P, dtype=np.float32).astype(npbf16),
        })
        perm_slots.append(sl)
    return in_maps, perm_slots


# ------------------------------------------------------------------ entry
def kernel(x_cat, edge_index, emb_tables, W1, b1, W2, b2,
           gamma1, beta1, gamma2, beta2, _res_hook=None):
    if "nc" not in _CACHE:
        _CACHE["nc"] = build_program()
    nc = _CACHE["nc"]
    in_maps, perm_slots = _host_prep(
        np.asarray(x_cat), np.asarray(edge_index), np.asarray(emb_tables),
        np.asarray(W1), np.asarray(b1), np.asarray(W2), np.asarray(b2),
        np.asarray(gamma1), np.asarray(beta1), np.asarray(gamma2),
        np.asarray(beta2))
    res = run_bass_kernel_spmd(nc, in_maps, list(range(NCORE)),
                               **(_res_hook or {}))
    out = np.empty((N, OUT), np.float32)
    for k in range(NCORE):
        full = res.results[k]["outx"]        # [SLOTS, OUT] slot-ordered
        out[k * SH:(k + 1) * SH] = full[perm_slots[k]]
    if _res_hook is not None:
        _res_hook["result"] = res
    return out
